# revision 9
# baseline (speedup 1.0000x reference)
"""Multi-Latent Attention TRN2 kernel.

Sharding: tensor-parallel over heads. 16 heads / 8 cores = 2 heads per core.
Each core computes its 2 heads' projections + attention and a partial of the
final output projection (contracting only its heads' feature block); the host
sums the 8 partials and adds the output bias.

On-device dataflow is feature-major (transposed): the host feeds X^T for
queries/keys/values so every matmul contracts along SBUF partitions.

  q^T   = Wq_c^T  X_q^T            [256, T]
  latk^T= Wlk_c^T X_k^T            [128, T]
  latv^T= Wlv_c^T X_v^T            [128, T]
  k^T   = blockdiag(Wkr)^T latk^T  [256, T]  (per head)
  v     = latv blockdiag(Wvr)      [T, 256]  (token-major)
  P~^T  = exp(k q^T / sqrt(dk))    (S^T computed directly; no transposes)
  rowsum= ones^T P~^T              (ones-vector matmul)
  U^T   = v^T P~^T
  attnout^T = U^T * recip(rowsum) + bvr
  out_partial = attnout @ Wo_rows

Softmax skips the max-subtraction: scores are O(1) by construction
(inputs ~N(0,1), 1/sqrt(fan_in)-scaled weights), so exp cannot overflow.

v3 scheduling:
 - software pipeline: step i = {prefetch x DMAs for step i+1, projection of
   chunk c, attention for q-block Q=c}. Causality makes Q=c legal (q block c
   attends keys 0..512(c+1)), so attention PE work hides the next chunk's
   HBM traffic, which is what bounds the projection phase.
 - both heads' score tiles share one [128,2,512] PSUM tile so exp runs as a
   single 1024-wide ACT instruction (amortizes the ~352-cycle ACT overhead)
 - PSUM: 'st' pool 2x[128,2,512] (scores / q-proj / out-proj), 'acc' pool
   4x[128,512] (latents, recon, rowsum + U accumulators) - exactly 8 banks
 - rowsum/attnV matmuls for k-tile j-1 are emitted after scores j so the PE
   queue never blocks on the exp of the current tile
 - DMA queues: sync = xq + even outp tiles, scalar(ACT) = xk + odd outp
   tiles, gpsimd = xv + weights. Inputs are emitted ahead of outputs.
 - reciprocal via the fast approx DVE op (18 bits, softmax doesn't care)
"""

import math
from contextlib import ExitStack

import numpy as np

import concourse.mybir as mybir
from concourse import bacc
from concourse.bass import ds, ts
from concourse.tile import TileContext

# Problem constants (hardcoded per contract).
B, S, D = 2, 2048, 2048
H, DK, DV, L = 16, 128, 128, 64
N_CORES = 8
HPC = H // N_CORES        # heads per core = 2
T = B * S                 # 4096 tokens
SB = S                    # tokens per batch
FPC = HPC * DK            # feature cols per core = 256
LPC = HPC * L             # latent cols per core = 128
KO = D // 128             # contraction k-tiles over D = 16
KG = 4                    # ko-group size for input streaming
NG = KO // KG             # ko-groups per chunk = 4
QT = SB // 128            # 128-row tiles per batch = 16
NQB = SB // 512           # 512-wide q blocks per batch = 4
CHUNK = 512
NCH = SB // CHUNK         # 4

F32 = mybir.dt.float32
F32R = mybir.dt.float32r
BF16 = mybir.dt.bfloat16

IN_DT = BF16
OUT_DT = BF16

INV_SQRT_DK = 1.0 / math.sqrt(DK)
EXPF = mybir.ActivationFunctionType.Exp
IDF = mybir.ActivationFunctionType.Identity


def build_kernel():
    nc = bacc.Bacc(trn_type="TRN2", debug=False, num_swdge_queues=2)

    # ---- DRAM I/O ----
    qT = nc.dram_tensor("qT", [D, T], IN_DT, kind="ExternalInput")
    kT = nc.dram_tensor("kT", [D, T], IN_DT, kind="ExternalInput")
    vT = nc.dram_tensor("vT", [D, T], IN_DT, kind="ExternalInput")
    wq = nc.dram_tensor("wq", [D, FPC], IN_DT, kind="ExternalInput")
    bq = nc.dram_tensor("bq", [FPC], F32, kind="ExternalInput")
    wlk = nc.dram_tensor("wlk", [D, LPC], IN_DT, kind="ExternalInput")
    blk = nc.dram_tensor("blk", [LPC], F32, kind="ExternalInput")
    wlv = nc.dram_tensor("wlv", [D, LPC], IN_DT, kind="ExternalInput")
    blv = nc.dram_tensor("blv", [LPC], F32, kind="ExternalInput")
    wkr2 = nc.dram_tensor("wkr2", [LPC, FPC], F32R, kind="ExternalInput")
    bkr = nc.dram_tensor("bkr", [DK], F32, kind="ExternalInput")
    wvr2 = nc.dram_tensor("wvr2", [LPC, FPC], F32R, kind="ExternalInput")
    bvr = nc.dram_tensor("bvr", [DV], F32, kind="ExternalInput")
    wo = nc.dram_tensor("wo", [FPC, D], BF16, kind="ExternalInput")
    outp = nc.dram_tensor("outp", [T, D], OUT_DT, kind="ExternalOutput")

    with TileContext(nc) as tc, ExitStack() as ctx:
        ec = ctx.enter_context
        consts = ec(tc.tile_pool(name="consts", bufs=1))
        persist = ec(tc.tile_pool(name="persist", bufs=1))
        xpool = ec(tc.tile_pool(name="xpool", bufs=15))
        latpool = ec(tc.tile_pool(name="latpool", bufs=3))
        ptpool = ec(tc.tile_pool(name="ptpool", bufs=6))
        statpool = ec(tc.tile_pool(name="statpool", bufs=4))
        opool = ec(tc.tile_pool(name="opool", bufs=3))
        psA = ec(tc.tile_pool(name="psA", bufs=2, space="PSUM"))
        psB = ec(tc.tile_pool(name="psB", bufs=4, space="PSUM"))

        qT_r = qT.rearrange("(ko p) t -> p ko t", p=128)
        kT_r = kT.rearrange("(ko p) t -> p ko t", p=128)
        vT_r = vT.rearrange("(ko p) t -> p ko t", p=128)

        def emit_xdma(b, c):
            """Queue the three input streams for chunk (b, c), split into
            ko-groups so consumers can start on the first slice."""
            t0 = b * SB + c * CHUNK
            tiles = {}
            for g in range(NG):
                xqt = xpool.tile([128, KG, CHUNK], IN_DT, tag="x",
                                 name=f"xq{b}{c}{g}")
                nc.sync.dma_start(xqt, qT_r[:, ds(g * KG, KG), ds(t0, CHUNK)])
                tiles[("q", g)] = xqt
            for g in range(NG):
                xkt = xpool.tile([128, KG, CHUNK], IN_DT, tag="x",
                                 name=f"xk{b}{c}{g}")
                nc.sync.dma_start(xkt, kT_r[:, ds(g * KG, KG), ds(t0, CHUNK)])
                tiles[("k", g)] = xkt
            for g in range(NG):
                xvt = xpool.tile([128, KG, CHUNK], IN_DT, tag="x",
                                 name=f"xv{b}{c}{g}")
                nc.gpsimd.dma_start(xvt, vT_r[:, ds(g * KG, KG), ds(t0, CHUNK)])
                tiles[("v", g)] = xvt
            return tiles

        # ---- weights / constants; wq + first q groups first for startup ----
        wq_r = wq.rearrange("(ko p) m -> p ko m", p=128)
        wq_sb = consts.tile([128, KO, FPC], IN_DT, tag="wq")
        x0q = []
        for g in range(NG):
            nc.sync.dma_start(
                wq_sb[:, ds(g * KG, KG), :], wq_r[:, ds(g * KG, KG), :])
            xqt = xpool.tile([128, KG, CHUNK], IN_DT, tag="x",
                             name=f"xq00{g}")
            nc.scalar.dma_start(xqt, qT_r[:, ds(g * KG, KG), ds(0, CHUNK)])
            x0q.append(xqt)
        bq_sb = consts.tile([128, HPC], F32, tag="bq")
        nc.gpsimd.dma_start(bq_sb, bq.rearrange("(m p) -> p m", p=128))

        wlk_sb = consts.tile([128, KO, LPC], IN_DT, tag="wlk")
        nc.gpsimd.dma_start(wlk_sb, wlk.rearrange("(ko p) m -> p ko m", p=128))
        blk_sb = consts.tile([128, 1], F32, tag="blk")
        nc.gpsimd.dma_start(blk_sb, blk[:, None])
        x0k = []
        for g in range(NG):
            xkt = xpool.tile([128, KG, CHUNK], IN_DT, tag="x",
                             name=f"xk00{g}")
            nc.sync.dma_start(xkt, kT_r[:, ds(g * KG, KG), ds(0, CHUNK)])
            x0k.append(xkt)

        wlv_sb = consts.tile([128, KO, LPC], IN_DT, tag="wlv")
        nc.gpsimd.dma_start(wlv_sb, wlv.rearrange("(ko p) m -> p ko m", p=128))
        blv_sb = consts.tile([128, 1], F32, tag="blv")
        nc.gpsimd.dma_start(blv_sb, blv[:, None])
        x0v = []
        for g in range(NG):
            xvt = xpool.tile([128, KG, CHUNK], IN_DT, tag="x",
                             name=f"xv00{g}")
            nc.gpsimd.dma_start(xvt, vT_r[:, ds(g * KG, KG), ds(0, CHUNK)])
            x0v.append(xvt)

        wkr2_sb = consts.tile([128, FPC], F32R, tag="wkr2")
        nc.gpsimd.dma_start(wkr2_sb, wkr2[:, :])
        wvr2_sb = consts.tile([128, FPC], F32R, tag="wvr2")
        nc.gpsimd.dma_start(wvr2_sb, wvr2[:, :])
        bkr_sb = consts.tile([128, 1], F32, tag="bkr")
        nc.gpsimd.dma_start(bkr_sb, bkr[:, None])
        bvr_sb = consts.tile([128, 1], F32, tag="bvr")
        nc.gpsimd.dma_start(bvr_sb, bvr[:, None])

        # causal mask for a diagonal 128x128 block of P~^T: 1 where k <= q
        maskT = consts.tile([128, 128], BF16, tag="maskT")
        nc.gpsimd.memset(maskT, 1.0)
        nc.gpsimd.affine_select(
            out=maskT, in_=maskT, compare_op=mybir.AluOpType.is_ge,
            fill=0.0, base=0, pattern=[[1, 128]], channel_multiplier=-1,
        )
        ones_bf = consts.tile([128, 128], BF16, tag="ones_bf")
        nc.gpsimd.memset(ones_bf, 1.0)

        wo_sb = consts.tile([128, HPC, D], BF16, tag="wo")
        nc.gpsimd.dma_start(wo_sb, wo.rearrange("(kk p) d -> p kk d", p=128))

        # attnout^T (both batches), feature-major, lhsT of final matmul
        asb = persist.tile([128, HPC, T], BF16, tag="asb")
        qsbs, ksbs, vsbs = [], [], []
        for b in range(B):
            qsb_ = persist.tile([128, HPC, SB], BF16, tag=f"qsb{b}",
                                name=f"qsb{b}")
            ksb_ = persist.tile([128, HPC, SB], BF16, tag=f"ksb{b}",
                                name=f"ksb{b}")
            vsb_ = persist.tile([128, QT, FPC], BF16, tag=f"vsb{b}",
                                name=f"vsb{b}")
            qsbs.append(qsb_)
            ksbs.append(ksb_)
            vsbs.append(vsb_)

        steps = [(b, c) for b in range(B) for c in range(NCH)]
        pending_outproj = None
        xcur = {}
        for g in range(NG):
            xcur[("q", g)] = x0q[g]
            xcur[("k", g)] = x0k[g]
            xcur[("v", g)] = x0v[g]

        for i, (b, c) in enumerate(steps):
            qsb, ksb, vsb = qsbs[b], ksbs[b], vsbs[b]
            if i + 1 < len(steps):
                xnext = emit_xdma(*steps[i + 1])
            else:
                xnext = None

            # ---- projection of chunk (b, c) ----
            csl = ds(c * CHUNK, CHUNK)

            stq = psA.tile([128, HPC, 512], F32, tag="st")
            for ko in range(KO):
                for m in range(HPC):
                    nc.tensor.matmul(
                        stq[:, m, :],
                        wq_sb[:, ko, ts(m, 128)],
                        xcur[("q", ko // KG)][:, ko % KG, :],
                        start=(ko == 0), stop=(ko == KO - 1),
                    )
            for m in range(HPC):
                nc.scalar.activation(
                    qsb[:, m, csl], stq[:, m, :], IDF,
                    bias=bq_sb[:, m : m + 1])

            psl = psB.tile([128, 512], F32, tag="acc")
            for ko in range(KO):
                nc.tensor.matmul(
                    psl, wlk_sb[:, ko, :], xcur[("k", ko // KG)][:, ko % KG, :],
                    start=(ko == 0), stop=(ko == KO - 1),
                )
            lk = latpool.tile([128, 512], F32R, tag="lat")
            nc.scalar.activation(lk, psl, IDF, bias=blk_sb[:, 0:1])
            for h in range(HPC):
                psk = psB.tile([128, 512], F32, tag="acc")
                nc.tensor.matmul(
                    psk, wkr2_sb[:, ts(h, 128)], lk, start=True, stop=True)
                nc.vector.tensor_scalar_add(
                    ksb[:, h, csl], psk, bkr_sb[:, 0:1])

            psv = psB.tile([128, 512], F32, tag="acc")
            for ko in range(KO):
                nc.tensor.matmul(
                    psv, wlv_sb[:, ko, :], xcur[("v", ko // KG)][:, ko % KG, :],
                    start=(ko == 0), stop=(ko == KO - 1),
                )
            lv = latpool.tile([128, 512], F32R, tag="lat")
            nc.scalar.activation(lv, psv, IDF, bias=blv_sb[:, 0:1])
            for jp in range(2):  # pairs of 128-token v tiles
                psu2 = psB.tile([128, 512], F32, tag="acc")
                for j2 in range(2):
                    nc.tensor.matmul(
                        psu2[:, ts(j2, 256)],
                        lv[:, ts(2 * jp + j2, 128)], wvr2_sb,
                        start=True, stop=True,
                    )
                jt = (c * CHUNK) // 128 + 2 * jp
                nc.any.tensor_copy(
                    out=vsb[:, ds(jt, 2), :],
                    in_=psu2.rearrange("p (a b) -> p a b", a=2),
                )

            if pending_outproj is not None:
                pending_outproj()
                pending_outproj = None

            # ---- attention for q-block Q = c ----
            Q = c
            jmax = 4 * Q + 4          # k-tiles 0..jmax-1
            o_acc = [psB.tile([128, 512], F32, tag="acc",
                              name=f"o_acc{h}") for h in range(HPC)]
            u_acc = [psB.tile([128, 512], F32, tag="acc",
                              name=f"u_acc{h}") for h in range(HPC)]
            pts = [None] * jmax

            def emit_ou(j):
                qoff, pt = pts[j]
                for h in range(HPC):
                    nc.tensor.matmul(
                        o_acc[h][:, qoff:], ones_bf, pt[:, h, qoff:],
                        start=(j == 0), stop=(j == jmax - 1),
                    )
                    nc.tensor.matmul(
                        u_acc[h][:, qoff:], vsb[:, j, ts(h, 128)],
                        pt[:, h, qoff:],
                        start=(j == 0), stop=(j == jmax - 1),
                    )

            for j in range(jmax):
                qoff = max(0, (j - 4 * Q) * 128)
                n = 512 - qoff
                st = psA.tile([128, HPC, 512], F32, tag="st")
                for h in range(HPC):
                    nc.tensor.matmul(
                        st[:, h, qoff:], ksb[:, h, ts(j, 128)],
                        qsb[:, h, ds(Q * 512 + qoff, n)],
                        start=True, stop=True,
                    )
                pt = ptpool.tile([128, HPC, 512], BF16, tag="pt")
                nc.scalar.activation(
                    pt[:, :, qoff:], st[:, :, qoff:],
                    EXPF, scale=INV_SQRT_DK,
                )
                if j >= 4 * Q:  # diagonal k-tile: causal mask
                    for h in range(HPC):
                        nc.vector.tensor_tensor(
                            pt[:, h, ds(qoff, 128)],
                            pt[:, h, ds(qoff, 128)],
                            maskT, mybir.AluOpType.mult,
                        )
                pts[j] = (qoff, pt)
                if j > 0:
                    emit_ou(j - 1)
            emit_ou(jmax - 1)

            for h in range(HPC):
                rcp_sb = statpool.tile([128, 512], F32, tag="rcp")
                nc.vector.reciprocal_approx_fast(rcp_sb, o_acc[h])
                a_sl = asb[:, h, ds(b * SB + Q * 512, 512)]
                nc.vector.tensor_tensor(a_sl, u_acc[h], rcp_sb,
                                        mybir.AluOpType.mult)
                nc.vector.tensor_scalar_add(a_sl, a_sl, bvr_sb[:, 0:1])

            # out-projection for this q-block, deferred into the next
            # step's projection window so it does not stall on the
            # normalize chain and scores of the next block do not stall
            # on its PSUM buffers
            def make_outproj(b, Q):
                def emit_outproj():
                    for tl in range(4):
                        tt = b * QT + Q * 4 + tl
                        o_sb = opool.tile([128, D], OUT_DT, tag="o")
                        for dc2 in range(2):
                            ps_f = psA.tile([128, HPC, 512], F32, tag="st")
                            for half in range(2):
                                dc = dc2 * 2 + half
                                for kk in range(HPC):
                                    nc.tensor.matmul(
                                        ps_f[:, half, :],
                                        asb[:, kk, ts(tt, 128)],
                                        wo_sb[:, kk, ts(dc, 512)],
                                        start=(kk == 0),
                                        stop=(kk == HPC - 1),
                                    )
                            nc.any.tensor_copy(
                                out=o_sb[:, ds(dc2 * 1024, 1024)].rearrange(
                                    "p (a b) -> p a b", a=2),
                                in_=ps_f,
                            )
                        nc.gpsimd.dma_start(outp[ts(tt, 128), :], o_sb)
                return emit_outproj

            pending_outproj = make_outproj(b, Q)
            xcur = xnext

        pending_outproj()

    nc.finalize()
    return nc


_NC_CACHE = None


def _get_nc():
    global _NC_CACHE
    if _NC_CACHE is None:
        _NC_CACHE = build_kernel()
    return _NC_CACHE


def _prep_in_maps(queries, keys, values, Wq, bq, Wlk, blk, Wlv, blv,
                  Wkr, bkr, Wvr, bvr, Wo, bo):
    f = np.float32
    import ml_dtypes

    ind = ml_dtypes.bfloat16

    qTh = np.ascontiguousarray(queries.reshape(T, D).T.astype(ind))
    kTh = np.ascontiguousarray(keys.reshape(T, D).T.astype(ind))
    vTh = np.ascontiguousarray(values.reshape(T, D).T.astype(ind))

    wkr2 = np.zeros((LPC, FPC), f)
    wkr2[0:L, 0:DK] = Wkr
    wkr2[L : 2 * L, DK : 2 * DK] = Wkr
    wvr2 = np.zeros((LPC, FPC), f)
    wvr2[0:L, 0:DV] = Wvr
    wvr2[L : 2 * L, DV : 2 * DV] = Wvr

    in_maps = []
    for c in range(N_CORES):
        fsl = slice(c * FPC, (c + 1) * FPC)   # feature cols (q/k heads)
        lsl = slice(c * LPC, (c + 1) * LPC)   # latent cols
        in_maps.append({
            "qT": qTh, "kT": kTh, "vT": vTh,
            "wq": np.ascontiguousarray(Wq[:, fsl].astype(ind)),
            "bq": np.ascontiguousarray(bq[fsl], f),
            "wlk": np.ascontiguousarray(Wlk[:, lsl].astype(ind)),
            "blk": np.ascontiguousarray(blk[lsl], f),
            "wlv": np.ascontiguousarray(Wlv[:, lsl].astype(ind)),
            "blv": np.ascontiguousarray(blv[lsl], f),
            "wkr2": wkr2, "bkr": np.ascontiguousarray(bkr, f),
            "wvr2": wvr2, "bvr": np.ascontiguousarray(bvr, f),
            "wo": np.ascontiguousarray(Wo[fsl, :].astype(ml_dtypes.bfloat16)),
        })
    return in_maps


def _assemble(results, bo):
    acc = np.zeros((T, D), np.float64)
    for rmap in results:
        acc += rmap["outp"].astype(np.float64)
    acc += np.asarray(bo).astype(np.float64)
    return acc.astype(np.float32).reshape(B, S, D)


def kernel(**inputs):
    from concourse.bass_utils import run_bass_kernel_spmd

    nc = _get_nc()
    in_maps = _prep_in_maps(**inputs)
    res = run_bass_kernel_spmd(
        nc, in_maps, core_ids=list(range(N_CORES)), trace=False
    )
    return _assemble(res.results, inputs["bo"])


if __name__ == "__main__":
    nc = build_kernel()
    print("built ok, instructions:", len(nc.inst_map))


# revision 10
# speedup vs baseline: 1.1872x; 1.1872x over previous
"""Multi-Latent Attention TRN2 kernel.

Sharding: tensor-parallel over heads. 16 heads / 8 cores = 2 heads per core.
Each core computes its 2 heads' projections + attention and a partial of the
final output projection (contracting only its heads' feature block); the host
sums the 8 partials and adds the output bias.

On-device dataflow is feature-major (transposed): the host feeds X^T for
queries/keys/values so every matmul contracts along SBUF partitions.

  q^T   = Wq_c^T  X_q^T            [256, T]
  latk^T= Wlk_c^T X_k^T            [128, T]
  latv^T= Wlv_c^T X_v^T            [128, T]
  k^T   = blockdiag(Wkr)^T latk^T  [256, T]  (per head)
  v     = latv blockdiag(Wvr)      [T, 256]  (token-major)
  P~^T  = exp(k q^T / sqrt(dk))    (S^T computed directly; no transposes)
  rowsum= ones^T P~^T              (ones-vector matmul)
  U^T   = v^T P~^T
  attnout^T = U^T * recip(rowsum) + bvr
  out_partial = attnout @ Wo_rows

Softmax skips the max-subtraction: scores are O(1) by construction
(inputs ~N(0,1), 1/sqrt(fan_in)-scaled weights), so exp cannot overflow.

v3 scheduling:
 - software pipeline: step i = {prefetch x DMAs for step i+1, projection of
   chunk c, attention for q-block Q=c}. Causality makes Q=c legal (q block c
   attends keys 0..512(c+1)), so attention PE work hides the next chunk's
   HBM traffic, which is what bounds the projection phase.
 - both heads' score tiles share one [128,2,512] PSUM tile so exp runs as a
   single 1024-wide ACT instruction (amortizes the ~352-cycle ACT overhead)
 - PSUM: 'st' pool 2x[128,2,512] (scores / q-proj / out-proj), 'acc' pool
   4x[128,512] (latents, recon, rowsum + U accumulators) - exactly 8 banks
 - rowsum/attnV matmuls for k-tile j-1 are emitted after scores j so the PE
   queue never blocks on the exp of the current tile
 - DMA queues: sync = xq + even outp tiles, scalar(ACT) = xk + odd outp
   tiles, gpsimd = xv + weights. Inputs are emitted ahead of outputs.
 - reciprocal via the fast approx DVE op (18 bits, softmax doesn't care)
"""

import math
from contextlib import ExitStack

import numpy as np

import concourse.mybir as mybir
from concourse import bacc
from concourse.bass import ds, ts
from concourse.tile import TileContext

# Problem constants (hardcoded per contract).
B, S, D = 2, 2048, 2048
H, DK, DV, L = 16, 128, 128, 64
N_CORES = 8
HPC = H // N_CORES        # heads per core = 2
T = B * S                 # 4096 tokens
SB = S                    # tokens per batch
FPC = HPC * DK            # feature cols per core = 256
LPC = HPC * L             # latent cols per core = 128
KO = D // 128             # contraction k-tiles over D = 16
KG = 4                    # ko-group size for input streaming
NG = KO // KG             # ko-groups per chunk = 4
QT = SB // 128            # 128-row tiles per batch = 16
NQB = SB // 512           # 512-wide q blocks per batch = 4
CHUNK = 512
NCH = SB // CHUNK         # 4

F32 = mybir.dt.float32
F32R = mybir.dt.float32r
BF16 = mybir.dt.bfloat16

IN_DT = BF16
OUT_DT = BF16

INV_SQRT_DK = 1.0 / math.sqrt(DK)
EXPF = mybir.ActivationFunctionType.Exp
IDF = mybir.ActivationFunctionType.Identity


def build_kernel():
    nc = bacc.Bacc(trn_type="TRN2", debug=False, num_swdge_queues=2)

    # ---- DRAM I/O ----
    qT = nc.dram_tensor("qT", [D, T], IN_DT, kind="ExternalInput")
    kT = nc.dram_tensor("kT", [D, T], IN_DT, kind="ExternalInput")
    vT = nc.dram_tensor("vT", [D, T], IN_DT, kind="ExternalInput")
    wq = nc.dram_tensor("wq", [D, FPC], IN_DT, kind="ExternalInput")
    bq = nc.dram_tensor("bq", [FPC], F32, kind="ExternalInput")
    wlk = nc.dram_tensor("wlk", [D, LPC], IN_DT, kind="ExternalInput")
    blk = nc.dram_tensor("blk", [LPC], F32, kind="ExternalInput")
    wlv = nc.dram_tensor("wlv", [D, LPC], IN_DT, kind="ExternalInput")
    blv = nc.dram_tensor("blv", [LPC], F32, kind="ExternalInput")
    wkr2 = nc.dram_tensor("wkr2", [LPC, FPC], F32R, kind="ExternalInput")
    bkr = nc.dram_tensor("bkr", [DK], F32, kind="ExternalInput")
    wvr2 = nc.dram_tensor("wvr2", [LPC, FPC], F32R, kind="ExternalInput")
    bvr = nc.dram_tensor("bvr", [DV], F32, kind="ExternalInput")
    wo = nc.dram_tensor("wo", [FPC, D], BF16, kind="ExternalInput")
    outp = nc.dram_tensor("outp", [T, D], OUT_DT, kind="ExternalOutput")

    with TileContext(nc) as tc, ExitStack() as ctx:
        ec = ctx.enter_context
        consts = ec(tc.tile_pool(name="consts", bufs=1))
        persist = ec(tc.tile_pool(name="persist", bufs=1))
        xpool = ec(tc.tile_pool(name="xpool", bufs=15))
        latpool = ec(tc.tile_pool(name="latpool", bufs=3))
        ptpool = ec(tc.tile_pool(name="ptpool", bufs=6))
        statpool = ec(tc.tile_pool(name="statpool", bufs=4))
        opool = ec(tc.tile_pool(name="opool", bufs=3))
        psA = ec(tc.tile_pool(name="psA", bufs=2, space="PSUM"))
        psB = ec(tc.tile_pool(name="psB", bufs=4, space="PSUM"))

        qT_r = qT.rearrange("(ko p) t -> p ko t", p=128)
        kT_r = kT.rearrange("(ko p) t -> p ko t", p=128)
        vT_r = vT.rearrange("(ko p) t -> p ko t", p=128)

        def emit_xdma(b, c):
            """Queue the three input streams for chunk (b, c), split into
            ko-groups so consumers can start on the first slice."""
            t0 = b * SB + c * CHUNK
            tiles = {}
            for g in range(NG):
                xqt = xpool.tile([128, KG, CHUNK], IN_DT, tag="x",
                                 name=f"xq{b}{c}{g}")
                nc.sync.dma_start(xqt, qT_r[:, ds(g * KG, KG), ds(t0, CHUNK)])
                tiles[("q", g)] = xqt
            for g in range(NG):
                xkt = xpool.tile([128, KG, CHUNK], IN_DT, tag="x",
                                 name=f"xk{b}{c}{g}")
                nc.sync.dma_start(xkt, kT_r[:, ds(g * KG, KG), ds(t0, CHUNK)])
                tiles[("k", g)] = xkt
            for g in range(NG):
                xvt = xpool.tile([128, KG, CHUNK], IN_DT, tag="x",
                                 name=f"xv{b}{c}{g}")
                nc.gpsimd.dma_start(xvt, vT_r[:, ds(g * KG, KG), ds(t0, CHUNK)])
                tiles[("v", g)] = xvt
            return tiles

        # ---- weights / constants; wq + first q groups first for startup ----
        wq_r = wq.rearrange("(ko p) m -> p ko m", p=128)
        wq_sb = consts.tile([128, KO, FPC], IN_DT, tag="wq")
        x0q = []
        for g in range(NG):
            nc.gpsimd.dma_start(
                wq_sb[:, ds(g * KG, KG), :], wq_r[:, ds(g * KG, KG), :])
            xqt = xpool.tile([128, KG, CHUNK], IN_DT, tag="x",
                             name=f"xq00{g}")
            nc.sync.dma_start(xqt, qT_r[:, ds(g * KG, KG), ds(0, CHUNK)])
            x0q.append(xqt)
        bq_sb = consts.tile([128, HPC], F32, tag="bq")
        nc.gpsimd.dma_start(bq_sb, bq.rearrange("(m p) -> p m", p=128))

        wlk_sb = consts.tile([128, KO, LPC], IN_DT, tag="wlk")
        nc.gpsimd.dma_start(wlk_sb, wlk.rearrange("(ko p) m -> p ko m", p=128))
        blk_sb = consts.tile([128, 1], F32, tag="blk")
        nc.gpsimd.dma_start(blk_sb, blk[:, None])
        x0k = []
        for g in range(NG):
            xkt = xpool.tile([128, KG, CHUNK], IN_DT, tag="x",
                             name=f"xk00{g}")
            nc.sync.dma_start(xkt, kT_r[:, ds(g * KG, KG), ds(0, CHUNK)])
            x0k.append(xkt)

        wlv_sb = consts.tile([128, KO, LPC], IN_DT, tag="wlv")
        nc.gpsimd.dma_start(wlv_sb, wlv.rearrange("(ko p) m -> p ko m", p=128))
        blv_sb = consts.tile([128, 1], F32, tag="blv")
        nc.gpsimd.dma_start(blv_sb, blv[:, None])
        x0v = []
        for g in range(NG):
            xvt = xpool.tile([128, KG, CHUNK], IN_DT, tag="x",
                             name=f"xv00{g}")
            nc.gpsimd.dma_start(xvt, vT_r[:, ds(g * KG, KG), ds(0, CHUNK)])
            x0v.append(xvt)

        wkr2_sb = consts.tile([128, FPC], F32R, tag="wkr2")
        nc.gpsimd.dma_start(wkr2_sb, wkr2[:, :])
        wvr2_sb = consts.tile([128, FPC], F32R, tag="wvr2")
        nc.gpsimd.dma_start(wvr2_sb, wvr2[:, :])
        bkr_sb = consts.tile([128, 1], F32, tag="bkr")
        nc.gpsimd.dma_start(bkr_sb, bkr[:, None])
        bvr_sb = consts.tile([128, 1], F32, tag="bvr")
        nc.gpsimd.dma_start(bvr_sb, bvr[:, None])

        # causal mask for a diagonal 128x128 block of P~^T: 1 where k <= q
        maskT = consts.tile([128, 128], BF16, tag="maskT")
        nc.gpsimd.memset(maskT, 1.0)
        nc.gpsimd.affine_select(
            out=maskT, in_=maskT, compare_op=mybir.AluOpType.is_ge,
            fill=0.0, base=0, pattern=[[1, 128]], channel_multiplier=-1,
        )
        ones_bf = consts.tile([128, 128], BF16, tag="ones_bf")
        nc.gpsimd.memset(ones_bf, 1.0)

        wo_sb = consts.tile([128, HPC, D], BF16, tag="wo")
        nc.gpsimd.dma_start(wo_sb, wo.rearrange("(kk p) d -> p kk d", p=128))

        # attnout^T (both batches), feature-major, lhsT of final matmul
        asb = persist.tile([128, HPC, T], BF16, tag="asb")
        qsbs, ksbs, vsbs = [], [], []
        for b in range(B):
            qsb_ = persist.tile([128, HPC, SB], BF16, tag=f"qsb{b}",
                                name=f"qsb{b}")
            ksb_ = persist.tile([128, HPC, SB], BF16, tag=f"ksb{b}",
                                name=f"ksb{b}")
            vsb_ = persist.tile([128, QT, FPC], BF16, tag=f"vsb{b}",
                                name=f"vsb{b}")
            qsbs.append(qsb_)
            ksbs.append(ksb_)
            vsbs.append(vsb_)

        steps = [(b, c) for b in range(B) for c in range(NCH)]
        xcur = {}
        for g in range(NG):
            xcur[("q", g)] = x0q[g]
            xcur[("k", g)] = x0k[g]
            xcur[("v", g)] = x0v[g]

        for i, (b, c) in enumerate(steps):
            qsb, ksb, vsb = qsbs[b], ksbs[b], vsbs[b]
            if i + 1 < len(steps):
                xnext = emit_xdma(*steps[i + 1])
            else:
                xnext = None

            # ---- projection of chunk (b, c) ----
            csl = ds(c * CHUNK, CHUNK)

            stq = psA.tile([128, HPC, 512], F32, tag="st")
            for ko in range(KO):
                for m in range(HPC):
                    nc.tensor.matmul(
                        stq[:, m, :],
                        wq_sb[:, ko, ts(m, 128)],
                        xcur[("q", ko // KG)][:, ko % KG, :],
                        start=(ko == 0), stop=(ko == KO - 1),
                    )
            for m in range(HPC):
                nc.scalar.activation(
                    qsb[:, m, csl], stq[:, m, :], IDF,
                    bias=bq_sb[:, m : m + 1])

            psl = psB.tile([128, 512], F32, tag="acc")
            for ko in range(KO):
                nc.tensor.matmul(
                    psl, wlk_sb[:, ko, :], xcur[("k", ko // KG)][:, ko % KG, :],
                    start=(ko == 0), stop=(ko == KO - 1),
                )
            lk = latpool.tile([128, 512], F32R, tag="lat")
            nc.scalar.activation(lk, psl, IDF, bias=blk_sb[:, 0:1])
            for h in range(HPC):
                psk = psB.tile([128, 512], F32, tag="acc")
                nc.tensor.matmul(
                    psk, wkr2_sb[:, ts(h, 128)], lk, start=True, stop=True)
                nc.vector.tensor_scalar_add(
                    ksb[:, h, csl], psk, bkr_sb[:, 0:1])

            psv = psB.tile([128, 512], F32, tag="acc")
            for ko in range(KO):
                nc.tensor.matmul(
                    psv, wlv_sb[:, ko, :], xcur[("v", ko // KG)][:, ko % KG, :],
                    start=(ko == 0), stop=(ko == KO - 1),
                )
            lv = latpool.tile([128, 512], F32R, tag="lat")
            nc.scalar.activation(lv, psv, IDF, bias=blv_sb[:, 0:1])
            for jp in range(2):  # pairs of 128-token v tiles
                psu2 = psB.tile([128, 512], F32, tag="acc")
                for j2 in range(2):
                    nc.tensor.matmul(
                        psu2[:, ts(j2, 256)],
                        lv[:, ts(2 * jp + j2, 128)], wvr2_sb,
                        start=True, stop=True,
                    )
                jt = (c * CHUNK) // 128 + 2 * jp
                nc.any.tensor_copy(
                    out=vsb[:, ds(jt, 2), :],
                    in_=psu2.rearrange("p (a b) -> p a b", a=2),
                )

            # ---- attention + out-projection for q-block Q = c ----
            Q = c
            jmax = 4 * Q + 4          # k-tiles 0..jmax-1
            o_acc = [psB.tile([128, 512], F32, tag="acc",
                              name=f"o_acc{h}") for h in range(HPC)]
            u_acc = [psB.tile([128, 512], F32, tag="acc",
                              name=f"u_acc{h}") for h in range(HPC)]
            pts = [None] * jmax

            def emit_ou(j):
                qoff, pt = pts[j]
                for h in range(HPC):
                    nc.tensor.matmul(
                        o_acc[h][:, qoff:], ones_bf, pt[:, h, qoff:],
                        start=(j == 0), stop=(j == jmax - 1),
                    )
                    nc.tensor.matmul(
                        u_acc[h][:, qoff:], vsb[:, j, ts(h, 128)],
                        pt[:, h, qoff:],
                        start=(j == 0), stop=(j == jmax - 1),
                    )

            for j in range(jmax):
                qoff = max(0, (j - 4 * Q) * 128)
                n = 512 - qoff
                st = psA.tile([128, HPC, 512], F32, tag="st")
                for h in range(HPC):
                    nc.tensor.matmul(
                        st[:, h, qoff:], ksb[:, h, ts(j, 128)],
                        qsb[:, h, ds(Q * 512 + qoff, n)],
                        start=True, stop=True,
                    )
                pt = ptpool.tile([128, HPC, 512], BF16, tag="pt")
                nc.scalar.activation(
                    pt[:, :, qoff:], st[:, :, qoff:],
                    EXPF, scale=INV_SQRT_DK,
                )
                if j >= 4 * Q:  # diagonal k-tile: causal mask
                    for h in range(HPC):
                        nc.vector.tensor_tensor(
                            pt[:, h, ds(qoff, 128)],
                            pt[:, h, ds(qoff, 128)],
                            maskT, mybir.AluOpType.mult,
                        )
                pts[j] = (qoff, pt)
                if j > 0:
                    emit_ou(j - 1)
            emit_ou(jmax - 1)

            for h in range(HPC):
                rcp_sb = statpool.tile([128, 512], F32, tag="rcp")
                nc.vector.reciprocal_approx_fast(rcp_sb, o_acc[h])
                a_sl = asb[:, h, ds(b * SB + Q * 512, 512)]
                nc.vector.tensor_tensor(a_sl, u_acc[h], rcp_sb,
                                        mybir.AluOpType.mult)
                nc.vector.tensor_scalar_add(a_sl, a_sl, bvr_sb[:, 0:1])

            # final projection for this q-block's 4 token tiles
            for tl in range(4):
                tt = b * QT + Q * 4 + tl
                o_sb = opool.tile([128, D], OUT_DT, tag="o")
                for dc2 in range(2):
                    ps_f = psA.tile([128, HPC, 512], F32, tag="st")
                    for half in range(2):
                        dc = dc2 * 2 + half
                        for kk in range(HPC):
                            nc.tensor.matmul(
                                ps_f[:, half, :],
                                asb[:, kk, ts(tt, 128)],
                                wo_sb[:, kk, ts(dc, 512)],
                                start=(kk == 0), stop=(kk == HPC - 1),
                            )
                    nc.any.tensor_copy(
                        out=o_sb[:, ds(dc2 * 1024, 1024)].rearrange(
                            "p (a b) -> p a b", a=2),
                        in_=ps_f,
                    )
                if tl % 2 == 0:
                    nc.sync.dma_start(outp[ts(tt, 128), :], o_sb)
                else:
                    nc.gpsimd.dma_start(outp[ts(tt, 128), :], o_sb)

            xcur = xnext

    nc.finalize()
    return nc


_NC_CACHE = None


def _get_nc():
    global _NC_CACHE
    if _NC_CACHE is None:
        _NC_CACHE = build_kernel()
    return _NC_CACHE


def _prep_in_maps(queries, keys, values, Wq, bq, Wlk, blk, Wlv, blv,
                  Wkr, bkr, Wvr, bvr, Wo, bo):
    f = np.float32
    import ml_dtypes

    ind = ml_dtypes.bfloat16

    qTh = np.ascontiguousarray(queries.reshape(T, D).T.astype(ind))
    kTh = np.ascontiguousarray(keys.reshape(T, D).T.astype(ind))
    vTh = np.ascontiguousarray(values.reshape(T, D).T.astype(ind))

    wkr2 = np.zeros((LPC, FPC), f)
    wkr2[0:L, 0:DK] = Wkr
    wkr2[L : 2 * L, DK : 2 * DK] = Wkr
    wvr2 = np.zeros((LPC, FPC), f)
    wvr2[0:L, 0:DV] = Wvr
    wvr2[L : 2 * L, DV : 2 * DV] = Wvr

    in_maps = []
    for c in range(N_CORES):
        fsl = slice(c * FPC, (c + 1) * FPC)   # feature cols (q/k heads)
        lsl = slice(c * LPC, (c + 1) * LPC)   # latent cols
        in_maps.append({
            "qT": qTh, "kT": kTh, "vT": vTh,
            "wq": np.ascontiguousarray(Wq[:, fsl].astype(ind)),
            "bq": np.ascontiguousarray(bq[fsl], f),
            "wlk": np.ascontiguousarray(Wlk[:, lsl].astype(ind)),
            "blk": np.ascontiguousarray(blk[lsl], f),
            "wlv": np.ascontiguousarray(Wlv[:, lsl].astype(ind)),
            "blv": np.ascontiguousarray(blv[lsl], f),
            "wkr2": wkr2, "bkr": np.ascontiguousarray(bkr, f),
            "wvr2": wvr2, "bvr": np.ascontiguousarray(bvr, f),
            "wo": np.ascontiguousarray(Wo[fsl, :].astype(ml_dtypes.bfloat16)),
        })
    return in_maps


def _assemble(results, bo):
    acc = np.zeros((T, D), np.float64)
    for rmap in results:
        acc += rmap["outp"].astype(np.float64)
    acc += np.asarray(bo).astype(np.float64)
    return acc.astype(np.float32).reshape(B, S, D)


def kernel(**inputs):
    from concourse.bass_utils import run_bass_kernel_spmd

    nc = _get_nc()
    in_maps = _prep_in_maps(**inputs)
    res = run_bass_kernel_spmd(
        nc, in_maps, core_ids=list(range(N_CORES)), trace=False
    )
    return _assemble(res.results, inputs["bo"])


if __name__ == "__main__":
    nc = build_kernel()
    print("built ok, instructions:", len(nc.inst_map))
